# revision 4
# baseline (speedup 1.0000x reference)
"""nn_Attention Trainium2 Bass kernel — data-parallel over batch on 8 NeuronCores.

Per core (one batch element): full attention
  qh = q@Wq + bq; kh = k@Wk + bk; vh = kh@Wv + bv
  scores = qh@kh.T  (+ mask -> -10000); probs = softmax(scores)
  out = (probs @ vh) @ Wo + bo

Key optimizations vs the naive plan:
  * k-compaction: masked-out k columns (mask==0) contribute nothing, so the
    host gathers only the unmasked columns (~1058 of 2048) and pads to
    LKC=1280.  khT/vh/scores/AV PE work drops by LKC/L = 37.5%.
  * No DRAM spills: khT (f32r) and vh (fp16) stay SBUF-resident; vh and the
    bias row are computed per 512-column block right after that block of khT.
  * qh is projected per q-block inside the main loop (qhT = Wq.T @ qT), so
    scores = qhT.T-tiles @ khT with khT as the wide moving operand.
  * fp16 for the probs/vh/Wo path (same PE speed as bf16, 8x better eps).
    Scores path stays f32r end-to-end.
  * Wq/Wo streamed in 128-col slices just in time (small SBUF footprint).
  * Per q-block emission order interleaves next-qt scores with transposes and
    splits AV into two half-width passes so the PE never waits on softmax.

Device-side algebra (per core):
  khT[h,l]   = Wk-tiles.T @ ktc          (+bk per-partition)      [f32r]
  vh[l,v]    = khT.T-tiles @ Wv          (no bias; bv folded into bo2) [fp16]
  bqRow[k]   = bq.T @ khT ; biasRep = broadcast(maskBias + bqRow)
  per q-block: qhT[h,q] = Wq-tiles.T @ qT                          [f32r]
  per q-tile:  scores = qhT.T-tiles @ khT  [+ biasRep]
               softmax rowwise (max, exp with accum-sum, reciprocal)
               probsT via PE transposes (fp16)
  outUT[h,q] = vh-tiles.T @ probsT                                 [fp16]
  finalT[v,q]= Wo-tiles.T @ outUT       (+bo2 per-partition)       [f32]
Host: out[b] = finalT.T ;  bo2 = bv@Wo + bo  (exact: probs rows sum to 1)
"""
import numpy as np

import concourse.bass as bass
import concourse.mybir as mybir
from concourse import bacc, tile
from concourse.bass_utils import run_bass_kernel_spmd
from concourse.masks import make_identity

B, L, D, H = 8, 2048, 1024, 1024
P = 128
F32 = mybir.dt.float32
F32R = mybir.dt.float32r
FP16 = mybir.dt.float16
AF = mybir.ActivationFunctionType
AX = mybir.AxisListType

QBLK = 512          # q columns per main-loop block
NQB = L // QBLK     # 4
DT = D // P         # 8 d tiles
HT = H // P         # 8 h tiles

LKC_DEFAULT = 1280  # padded compact-k length (multiple of 256)


def build_nc(lkc=LKC_DEFAULT):
    assert lkc % 256 == 0
    # k-block slices: all >=256 wide so f32r matmuls stay 1 cycle/row
    kslices = []
    off = 0
    while off < lkc:
        w = 512 if lkc - off >= 512 else lkc - off
        kslices.append((off, w))
        off += w
    nkt = lkc // P  # k tiles of 128

    nc = bacc.Bacc("TRN2", target_bir_lowering=False, debug=False, num_devices=8)
    qt_d = nc.dram_tensor("qt", [D, L], F32R, kind="ExternalInput").ap()
    ktc_d = nc.dram_tensor("ktc", [D, lkc], F32R, kind="ExternalInput").ap()
    wk_d = nc.dram_tensor("wk", [D, H], F32R, kind="ExternalInput").ap()
    wq_d = nc.dram_tensor("wq", [D, H], F32R, kind="ExternalInput").ap()
    wv_d = nc.dram_tensor("wv", [H, H], FP16, kind="ExternalInput").ap()
    wo_d = nc.dram_tensor("wo", [H, D], FP16, kind="ExternalInput").ap()
    bk_d = nc.dram_tensor("bk", [H, 1], F32, kind="ExternalInput").ap()
    bq_d = nc.dram_tensor("bq", [H, 1], F32R, kind="ExternalInput").ap()
    bo2_d = nc.dram_tensor("bo2", [D, 1], F32, kind="ExternalInput").ap()
    maskb_d = nc.dram_tensor("maskb", [1, lkc], F32, kind="ExternalInput").ap()
    ones_d = nc.dram_tensor("ones", [1, P], F32R, kind="ExternalInput").ap()
    out_d = nc.dram_tensor("out", [D, L], F32, kind="ExternalOutput").ap()

    with tile.TileContext(nc) as tc:
        with tc.tile_pool(name="const", bufs=1) as cp, \
             tc.tile_pool(name="persist", bufs=1) as pp_:
            bk_t = cp.tile([P, HT], F32)
            bq_t = cp.tile([P, HT], F32R)
            bo2_t = cp.tile([P, DT], F32)
            for i in range(HT):
                nc.gpsimd.dma_start(out=bk_t[:, i:i + 1], in_=bk_d[i * P:(i + 1) * P, :])
                nc.gpsimd.dma_start(out=bq_t[:, i:i + 1], in_=bq_d[i * P:(i + 1) * P, :])
                nc.gpsimd.dma_start(out=bo2_t[:, i:i + 1], in_=bo2_d[i * P:(i + 1) * P, :])
            onesr_t = cp.tile([1, P], F32R)
            nc.gpsimd.dma_start(out=onesr_t, in_=ones_d)
            mrow_t = cp.tile([1, lkc], F32)
            nc.gpsimd.dma_start(out=mrow_t, in_=maskb_d)
            ident_f = cp.tile([P, P], F32)
            make_identity(nc, ident_f)
            identh_t = cp.tile([P, P], FP16)
            nc.vector.tensor_copy(identh_t, ident_f)
            biasrep_t = cp.tile([P, lkc], F32)

            # persistent across the whole kernel
            khb_r = pp_.tile([P, HT, lkc], F32R)    # khT, h on partitions
            vh_t = pp_.tile([P, nkt, H], FP16)      # vh,  k on partitions

            # ---------- A: khT (+bk), vh, bias row — fused per k-block
            with tc.tile_pool(name="aw", bufs=1) as awp, \
                 tc.tile_pool(name="akt", bufs=2) as aktp, \
                 tc.tile_pool(name="akh", bufs=2) as akhp, \
                 tc.tile_pool(name="aps", bufs=2, space="PSUM") as apsp, \
                 tc.tile_pool(name="abps", bufs=1, space="PSUM") as abpsp:
                wk_t = awp.tile([P, DT, H], F32R, tag="wk")
                wv_t = awp.tile([P, HT, H], FP16, tag="wv")
                for i in range(HT):
                    nc.scalar.dma_start(out=wv_t[:, i], in_=wv_d[i * P:(i + 1) * P, :])
                for lb, (koff, kw) in enumerate(kslices):
                    ktb = aktp.tile([P, DT, 512], F32R, tag="ktb")
                    for d in range(DT):
                        if lb == 0:
                            nc.sync.dma_start(out=wk_t[:, d],
                                              in_=wk_d[d * P:(d + 1) * P, :])
                        nc.sync.dma_start(out=ktb[:, d, 0:kw],
                                          in_=ktc_d[d * P:(d + 1) * P, koff:koff + kw])
                    khh = akhp.tile([P, HT, 512], FP16, tag="khh")
                    for ht in range(HT):
                        ps = apsp.tile([P, 512], F32, tag="ps")
                        for d in range(DT):
                            nc.tensor.matmul(ps[:, 0:kw],
                                             wk_t[:, d, ht * P:(ht + 1) * P],
                                             ktb[:, d, 0:kw],
                                             start=(d == 0), stop=(d == DT - 1))
                        nc.scalar.activation(khb_r[:, ht, koff:koff + kw], ps[:, 0:kw],
                                             AF.Identity, bias=bk_t[:, ht:ht + 1])
                        nc.vector.tensor_copy(khh[:, ht, 0:kw],
                                              khb_r[:, ht, koff:koff + kw])
                    # vh for this k-block
                    for lt in range(kw // P):
                        kt_idx = (koff // P) + lt
                        for vb in range(2):
                            ps = apsp.tile([P, 512], F32, tag="ps")
                            for h in range(HT):
                                nc.tensor.matmul(
                                    ps, khh[:, h, lt * P:(lt + 1) * P],
                                    wv_t[:, h, vb * 512:(vb + 1) * 512],
                                    start=(h == 0), stop=(h == HT - 1))
                            nc.scalar.activation(
                                vh_t[:, kt_idx, vb * 512:(vb + 1) * 512], ps, AF.Copy)
                    # bias row for this k-block: bqRow + mask, replicated
                    ps1 = abpsp.tile([1, 512], F32, tag="bqps")
                    for h in range(HT):
                        nc.tensor.matmul(ps1[:, 0:kw], bq_t[:, h:h + 1],
                                         khb_r[:, h, koff:koff + kw],
                                         start=(h == 0), stop=(h == HT - 1))
                    brow = akhp.tile([1, 512], F32, tag="brow", bufs=1)
                    nc.scalar.activation(brow[:, 0:kw], ps1[:, 0:kw], AF.Copy)
                    nc.vector.tensor_add(brow[:, 0:kw], brow[:, 0:kw],
                                         mrow_t[:, koff:koff + kw])
                    browr = akhp.tile([1, 512], F32R, tag="browr", bufs=1)
                    nc.vector.tensor_copy(browr[:, 0:kw], brow[:, 0:kw])
                    ps2 = apsp.tile([P, 512], F32, tag="ps")
                    nc.tensor.matmul(ps2[:, 0:kw], onesr_t, browr[:, 0:kw],
                                     start=True, stop=True)
                    nc.scalar.activation(biasrep_t[:, koff:koff + kw], ps2[:, 0:kw],
                                         AF.Copy)

            # ---------- B: attention per q-block
            with tc.tile_pool(name="bq2", bufs=2) as qp, \
                 tc.tile_pool(name="bwq", bufs=2) as wqp, \
                 tc.tile_pool(name="bwo", bufs=2) as wop, \
                 tc.tile_pool(name="bqh", bufs=1) as qhp, \
                 tc.tile_pool(name="bsm", bufs=2) as smp, \
                 tc.tile_pool(name="bpt", bufs=1) as ptp, \
                 tc.tile_pool(name="bou", bufs=1) as oup, \
                 tc.tile_pool(name="bst", bufs=2) as stp, \
                 tc.tile_pool(name="bps_s", bufs=2, space="PSUM") as pss, \
                 tc.tile_pool(name="bps_m", bufs=2, space="PSUM") as psm:
                qtbs = {}
                for qb in range(NQB):
                    if qb == 0:
                        qtb = qp.tile([P, DT, QBLK], F32R, tag="qtb")
                        for d in range(DT):
                            nc.sync.dma_start(out=qtb[:, d],
                                              in_=qt_d[d * P:(d + 1) * P, 0:QBLK])
                    else:
                        qtb = qtbs.pop(qb)
                    # prefetch next q-block
                    if qb + 1 < NQB:
                        nxq = qp.tile([P, DT, QBLK], F32R, tag="qtb")
                        for d in range(DT):
                            nc.sync.dma_start(
                                out=nxq[:, d],
                                in_=qt_d[d * P:(d + 1) * P,
                                         (qb + 1) * QBLK:(qb + 2) * QBLK])
                        qtbs[qb + 1] = nxq

                    # qhT = Wq.T-tiles @ qT   [h on partitions]
                    qhT = qhp.tile([P, HT, QBLK], F32R, tag="qhT")
                    for ht in range(HT):
                        wq_sl = wqp.tile([P, DT, P], F32R, tag="wq")
                        for d in range(DT):
                            nc.scalar.dma_start(
                                out=wq_sl[:, d],
                                in_=wq_d[d * P:(d + 1) * P, ht * P:(ht + 1) * P])
                        ps = psm.tile([P, QBLK], F32, tag="mmps")
                        for d in range(DT):
                            nc.tensor.matmul(ps, wq_sl[:, d], qtb[:, d],
                                             start=(d == 0), stop=(d == DT - 1))
                        nc.scalar.activation(qhT[:, ht], ps, AF.Copy)

                    probsT = ptp.tile([P, nkt, QBLK], FP16, tag="probsT")

                    def scores_softmax(qt):
                        sps = pss.tile([P, lkc], F32, tag="scps")
                        for (koff, kw) in kslices:
                            for h in range(HT):
                                nc.tensor.matmul(
                                    sps[:, koff:koff + kw],
                                    qhT[:, h, qt * P:(qt + 1) * P],
                                    khb_r[:, h, koff:koff + kw],
                                    start=(h == 0), stop=(h == HT - 1))
                        s_t = smp.tile([P, lkc], F32, tag="s", bufs=2)
                        nc.vector.tensor_add(s_t, sps, biasrep_t)
                        mx = smp.tile([P, 1], F32, tag="mx")
                        nc.vector.reduce_max(mx, s_t, axis=AX.X)
                        negmx = smp.tile([P, 1], F32, tag="negmx")
                        nc.vector.tensor_scalar_mul(negmx, mx, -1.0)
                        p_t = smp.tile([P, lkc], FP16, tag="pp", bufs=2)
                        sume = smp.tile([P, 1], F32, tag="sume")
                        nc.scalar.activation(p_t, s_t, AF.Exp, bias=negmx, scale=1.0,
                                             accum_out=sume)
                        recip = smp.tile([P, 1], F32, tag="recip")
                        nc.vector.reciprocal(recip, sume)
                        nc.vector.tensor_scalar_mul(p_t, p_t, recip)
                        return p_t

                    def transposes(qt, p_t):
                        for kt in range(nkt):
                            tp = psm.tile([P, P], FP16, tag="mmps")
                            nc.tensor.transpose(tp, p_t[:, kt * P:(kt + 1) * P],
                                                identh_t)
                            nc.scalar.activation(
                                probsT[:, kt, qt * P:(qt + 1) * P], tp, AF.Copy)

                    def av_half(outut, half):
                        q0 = half * (QBLK // 2)
                        qw = QBLK // 2
                        for ht in range(HT):
                            ps = psm.tile([P, QBLK], F32, tag="mmps")
                            for kt in range(nkt):
                                nc.tensor.matmul(
                                    ps[:, 0:qw],
                                    vh_t[:, kt, ht * P:(ht + 1) * P],
                                    probsT[:, kt, q0:q0 + qw],
                                    start=(kt == 0), stop=(kt == nkt - 1))
                            nc.scalar.activation(outut[:, ht, q0:q0 + qw],
                                                 ps[:, 0:qw], AF.Copy)

                    # emission order keeps the PE fed across softmax chains
                    p0 = scores_softmax(0)
                    p1 = scores_softmax(1)
                    transposes(0, p0)
                    p2 = scores_softmax(2)
                    transposes(1, p1)
                    p3 = scores_softmax(3)
                    outut = oup.tile([P, HT, QBLK], FP16, tag="outut")
                    av_half(outut, 0)
                    transposes(2, p2)
                    transposes(3, p3)
                    av_half(outut, 1)

                    # final projection (+bo2), write out
                    for vt in range(DT):
                        wo_sl = wop.tile([P, HT, P], FP16, tag="wo")
                        for h in range(HT):
                            nc.scalar.dma_start(
                                out=wo_sl[:, h],
                                in_=wo_d[h * P:(h + 1) * P, vt * P:(vt + 1) * P])
                        ps = psm.tile([P, QBLK], F32, tag="mmps")
                        for h in range(HT):
                            nc.tensor.matmul(ps, wo_sl[:, h], outut[:, h],
                                             start=(h == 0), stop=(h == HT - 1))
                        ot = stp.tile([P, QBLK], F32, tag="ot")
                        nc.scalar.activation(ot, ps, AF.Identity, bias=bo2_t[:, vt:vt + 1])
                        nc.sync.dma_start(
                            out=out_d[vt * P:(vt + 1) * P, qb * QBLK:(qb + 1) * QBLK],
                            in_=ot)
    nc.compile()
    return nc


_NC_CACHE = {}


def _get_nc(lkc=LKC_DEFAULT):
    if lkc not in _NC_CACHE:
        _NC_CACHE[lkc] = build_nc(lkc)
    return _NC_CACHE[lkc]


def _pad_up(n, m):
    return ((n + m - 1) // m) * m


def prepare(q, k, mask, Wq, bq, Wk, bk, Wv, bv, Wo, bo):
    """Returns (nc, in_maps) for run_bass_kernel_spmd."""
    q = np.asarray(q, np.float32)
    k = np.asarray(k, np.float32)
    mask = np.asarray(mask)
    Wq = np.asarray(Wq, np.float32)
    Wk = np.asarray(Wk, np.float32)
    Wv = np.asarray(Wv, np.float32)
    Wo = np.asarray(Wo, np.float32)
    bq_ = np.asarray(bq, np.float32)
    bk_ = np.asarray(bk, np.float32)
    bv_ = np.asarray(bv, np.float32)
    bo_ = np.asarray(bo, np.float32)

    nnz_max = int(mask.astype(bool).sum(axis=1).max())
    lkc = max(LKC_DEFAULT, _pad_up(max(nnz_max, 256), 256))
    nc = _get_nc(lkc)

    wq16 = Wq  # stays f32
    wv16 = Wv.astype(np.float16)
    wo16 = Wo.astype(np.float16)
    bo2 = (bv_.astype(np.float64) @ Wo.astype(np.float64) + bo_).astype(np.float32)
    ones = np.ones((1, P), np.float32)
    in_maps = []
    for b in range(B):
        idx = np.nonzero(mask[b])[0]
        ktc = np.zeros((D, lkc), np.float32)
        ktc[:, :len(idx)] = k[b][idx].T
        maskb = np.full((1, lkc), -10000.0, np.float32)
        maskb[0, :len(idx)] = 0.0
        in_maps.append({
            "qt": np.ascontiguousarray(q[b].T),
            "ktc": ktc,
            "wk": Wk, "wq": wq16, "wv": wv16, "wo": wo16,
            "bk": bk_.reshape(H, 1), "bq": bq_.reshape(H, 1),
            "bo2": bo2.reshape(D, 1),
            "maskb": maskb,
            "ones": ones,
        })
    return nc, in_maps


def kernel(q, k, mask, Wq, bq, Wk, bk, Wv, bv, Wo, bo):
    nc, in_maps = prepare(q, k, mask, Wq, bq, Wk, bk, Wv, bv, Wo, bo)
    res = run_bass_kernel_spmd(nc, in_maps, core_ids=list(range(B)))
    out = np.stack([np.ascontiguousarray(res.results[b]["out"].T) for b in range(B)])
    return out.astype(np.float32)


# revision 10
# speedup vs baseline: 1.6028x; 1.6028x over previous
"""nn_Attention Trainium2 Bass kernel — data-parallel over batch on 8 NeuronCores.

Per core (one batch element): full attention
  qh = q@Wq + bq; kh = k@Wk + bk; vh = kh@Wv + bv
  scores = qh@kh.T  (+ mask -> -10000); probs = softmax(scores)
  out = (probs @ vh) @ Wo + bo

Key optimizations vs the naive plan:
  * k-compaction: masked-out k columns (mask==0) contribute nothing, so the
    host gathers only the unmasked columns (~1058 of 2048) and pads to
    LKC=1280.  khT/vh/scores/AV PE work drops by LKC/L = 37.5%.
  * No DRAM spills: khT (f32r) and vh (fp16) stay SBUF-resident; vh and the
    bias row are computed per 512-column block right after that block of khT.
  * qh is projected per q-block inside the main loop (qhT = Wq.T @ qT), so
    scores = qhT.T-tiles @ khT with khT as the wide moving operand.
  * fp16 for the probs/vh/Wo path (same PE speed as bf16, 8x better eps).
    Scores path stays f32r end-to-end.
  * Wq/Wo streamed in 128-col slices just in time (small SBUF footprint).
  * Per q-block emission order interleaves next-qt scores with transposes and
    splits AV into two half-width passes so the PE never waits on softmax.

Device-side algebra (per core):
  khT[h,l]   = Wk-tiles.T @ ktc          (+bk per-partition)      [f32r]
  vh[l,v]    = khT.T-tiles @ Wv          (no bias; bv folded into bo2) [fp16]
  bqRow[k]   = bq.T @ khT ; biasRep = broadcast(maskBias + bqRow)
  per q-block: qhT[h,q] = Wq-tiles.T @ qT                          [f32r]
  per q-tile:  scores = qhT.T-tiles @ khT  [+ biasRep]
               softmax rowwise (max, exp with accum-sum, reciprocal)
               probsT via PE transposes (fp16)
  outUT[h,q] = vh-tiles.T @ probsT                                 [fp16]
  finalT[v,q]= Wo-tiles.T @ outUT       (+bo2 per-partition)       [f32]
Host: out[b] = finalT.T ;  bo2 = bv@Wo + bo  (exact: probs rows sum to 1)
"""
import numpy as np

import concourse.bass as bass
import concourse.mybir as mybir
from concourse import bacc, tile
from concourse.bass_utils import run_bass_kernel_spmd
from concourse.masks import make_identity

B, L, D, H = 8, 2048, 1024, 1024
P = 128
F32 = mybir.dt.float32
F32R = mybir.dt.float32r
FP16 = mybir.dt.float16
AF = mybir.ActivationFunctionType
AX = mybir.AxisListType

QBLK = 512          # q columns per main-loop block
NQB = L // QBLK     # 4
DT = D // P         # 8 d tiles
HT = H // P         # 8 h tiles

LKC_DEFAULT = 1280  # padded compact-k length (multiple of 256)


def build_nc(lkc=LKC_DEFAULT):
    assert lkc % 256 == 0
    # k-block slices: all >=256 wide so f32r matmuls stay 1 cycle/row
    kslices = []
    off = 0
    while off < lkc:
        w = 512 if lkc - off >= 512 else lkc - off
        kslices.append((off, w))
        off += w
    nkt = lkc // P  # k tiles of 128

    nc = bacc.Bacc("TRN2", target_bir_lowering=False, debug=False, num_devices=8)
    qt_d = nc.dram_tensor("qt", [D, L], F32R, kind="ExternalInput").ap()
    ktc_d = nc.dram_tensor("ktc", [D, lkc], F32R, kind="ExternalInput").ap()
    wk_d = nc.dram_tensor("wk", [D, H], F32R, kind="ExternalInput").ap()
    wq_d = nc.dram_tensor("wq", [D, H], F32R, kind="ExternalInput").ap()
    wv_d = nc.dram_tensor("wv", [H, H], FP16, kind="ExternalInput").ap()
    wo_d = nc.dram_tensor("wo", [H, D], FP16, kind="ExternalInput").ap()
    bk_d = nc.dram_tensor("bk", [H, 1], F32, kind="ExternalInput").ap()
    bq_d = nc.dram_tensor("bq", [H, 1], F32R, kind="ExternalInput").ap()
    bo2_d = nc.dram_tensor("bo2", [D, 1], F32, kind="ExternalInput").ap()
    maskb_d = nc.dram_tensor("maskb", [1, lkc], F32, kind="ExternalInput").ap()
    ones_d = nc.dram_tensor("ones", [1, P], F32R, kind="ExternalInput").ap()
    out_d = nc.dram_tensor("out", [D, L], F32, kind="ExternalOutput").ap()

    with tile.TileContext(nc) as tc:
        with tc.tile_pool(name="const", bufs=1) as cp, \
             tc.tile_pool(name="persist", bufs=1) as pp_:
            bk_t = cp.tile([P, HT], F32)
            bq_t = cp.tile([P, HT], F32R)
            bo2_t = cp.tile([P, DT], F32)
            for i in range(HT):
                nc.gpsimd.dma_start(out=bk_t[:, i:i + 1], in_=bk_d[i * P:(i + 1) * P, :])
                nc.gpsimd.dma_start(out=bq_t[:, i:i + 1], in_=bq_d[i * P:(i + 1) * P, :])
                nc.gpsimd.dma_start(out=bo2_t[:, i:i + 1], in_=bo2_d[i * P:(i + 1) * P, :])
            onesr_t = cp.tile([1, P], F32R)
            nc.gpsimd.dma_start(out=onesr_t, in_=ones_d)
            mrow_t = cp.tile([1, lkc], F32)
            nc.gpsimd.dma_start(out=mrow_t, in_=maskb_d)
            ident_f = cp.tile([P, P], F32)
            make_identity(nc, ident_f)
            identh_t = cp.tile([P, P], FP16)
            nc.vector.tensor_copy(identh_t, ident_f)
            biasrep_t = cp.tile([P, lkc], F32)

            # persistent across the whole kernel.  khT is split into two
            # tiles so every 512-col (and the 256-col) moving slice is
            # 2KB-aligned in SBUF (a 5120B row stride would misalign odd-h
            # slices and halve the PE stream rate).
            khbA = pp_.tile([P, HT, 1024], F32R)    # khT k-cols [0,1024)
            khbB = pp_.tile([P, HT, lkc - 1024], F32R)  # khT k-cols [1024,lkc)
            vh_t = pp_.tile([P, nkt, H], FP16)      # vh,  k on partitions

            def khb_slice(h, koff, kw):
                if koff < 1024:
                    assert koff + kw <= 1024
                    return khbA[:, h, koff:koff + kw]
                return khbB[:, h, koff - 1024:koff - 1024 + kw]

            # ---------- A: khT (+bk), vh, bias row — fused per k-block
            with tc.tile_pool(name="aw", bufs=1) as awp, \
                 tc.tile_pool(name="akt", bufs=2) as aktp, \
                 tc.tile_pool(name="akh", bufs=2) as akhp, \
                 tc.tile_pool(name="aps", bufs=2, space="PSUM") as apsp, \
                 tc.tile_pool(name="abps", bufs=1, space="PSUM") as abpsp:
                wk_t = awp.tile([P, DT, H], F32R, tag="wk")
                wv_t = awp.tile([P, HT, H], FP16, tag="wv")
                for i in range(HT):
                    nc.scalar.dma_start(out=wv_t[:, i], in_=wv_d[i * P:(i + 1) * P, :])
                for lb, (koff, kw) in enumerate(kslices):
                    ktb = aktp.tile([P, DT, 512], F32R, tag="ktb")
                    for d in range(DT):
                        if lb == 0:
                            nc.sync.dma_start(out=wk_t[:, d],
                                              in_=wk_d[d * P:(d + 1) * P, :])
                        nc.sync.dma_start(out=ktb[:, d, 0:kw],
                                          in_=ktc_d[d * P:(d + 1) * P, koff:koff + kw])
                    khh = akhp.tile([P, HT, 512], FP16, tag="khh")
                    for ht in range(HT):
                        ps = apsp.tile([P, 512], F32, tag="ps")
                        for d in range(DT):
                            nc.tensor.matmul(ps[:, 0:kw],
                                             wk_t[:, d, ht * P:(ht + 1) * P],
                                             ktb[:, d, 0:kw],
                                             start=(d == 0), stop=(d == DT - 1))
                        nc.scalar.activation(khb_slice(ht, koff, kw), ps[:, 0:kw],
                                             AF.Identity, bias=bk_t[:, ht:ht + 1])
                        nc.vector.tensor_copy(khh[:, ht, 0:kw],
                                              khb_slice(ht, koff, kw))
                    # vh for this k-block
                    for lt in range(kw // P):
                        kt_idx = (koff // P) + lt
                        for vb in range(2):
                            ps = apsp.tile([P, 512], F32, tag="ps")
                            for h in range(HT):
                                nc.tensor.matmul(
                                    ps, khh[:, h, lt * P:(lt + 1) * P],
                                    wv_t[:, h, vb * 512:(vb + 1) * 512],
                                    start=(h == 0), stop=(h == HT - 1))
                            nc.scalar.activation(
                                vh_t[:, kt_idx, vb * 512:(vb + 1) * 512], ps, AF.Copy)
                    # bias row for this k-block: bqRow + mask, replicated
                    ps1 = abpsp.tile([1, 512], F32, tag="bqps")
                    for h in range(HT):
                        nc.tensor.matmul(ps1[:, 0:kw], bq_t[:, h:h + 1],
                                         khb_slice(h, koff, kw),
                                         start=(h == 0), stop=(h == HT - 1))
                    brow = akhp.tile([1, 512], F32, tag="brow", bufs=1)
                    nc.scalar.activation(brow[:, 0:kw], ps1[:, 0:kw], AF.Copy)
                    nc.vector.tensor_add(brow[:, 0:kw], brow[:, 0:kw],
                                         mrow_t[:, koff:koff + kw])
                    browr = akhp.tile([1, 512], F32R, tag="browr", bufs=1)
                    nc.vector.tensor_copy(browr[:, 0:kw], brow[:, 0:kw])
                    ps2 = apsp.tile([P, 512], F32, tag="ps")
                    nc.tensor.matmul(ps2[:, 0:kw], onesr_t, browr[:, 0:kw],
                                     start=True, stop=True)
                    nc.scalar.activation(biasrep_t[:, koff:koff + kw], ps2[:, 0:kw],
                                         AF.Copy)

            # ---------- B: attention per q-block
            with tc.tile_pool(name="bq2", bufs=2) as qp, \
                 tc.tile_pool(name="bwq", bufs=1) as wqp, \
                 tc.tile_pool(name="bwo", bufs=1) as wop, \
                 tc.tile_pool(name="bqh", bufs=1) as qhp, \
                 tc.tile_pool(name="bsm", bufs=2) as smp, \
                 tc.tile_pool(name="bpt", bufs=1) as ptp, \
                 tc.tile_pool(name="bou", bufs=1) as oup, \
                 tc.tile_pool(name="bst", bufs=2) as stp, \
                 tc.tile_pool(name="bps_s", bufs=2, space="PSUM") as pss, \
                 tc.tile_pool(name="bps_m", bufs=2, space="PSUM") as psm:
                # Wq/Wo resident for all of B; the DMAs reuse SBUF freed by
                # the A-phase weight pools and overlap A's compute tail.
                wq_t = wqp.tile([P, DT, H], F32R, tag="wq")
                wo_t = wop.tile([P, HT, H], FP16, tag="wo")
                for i in range(DT):
                    nc.scalar.dma_start(out=wq_t[:, i], in_=wq_d[i * P:(i + 1) * P, :])
                for i in range(HT):
                    nc.scalar.dma_start(out=wo_t[:, i], in_=wo_d[i * P:(i + 1) * P, :])
                qtbs = {}
                for qb in range(NQB):
                    if qb == 0:
                        qtb = qp.tile([P, DT, QBLK], F32R, tag="qtb")
                        for d in range(DT):
                            nc.sync.dma_start(out=qtb[:, d],
                                              in_=qt_d[d * P:(d + 1) * P, 0:QBLK])
                    else:
                        qtb = qtbs.pop(qb)
                    # prefetch next q-block
                    if qb + 1 < NQB:
                        nxq = qp.tile([P, DT, QBLK], F32R, tag="qtb")
                        for d in range(DT):
                            nc.sync.dma_start(
                                out=nxq[:, d],
                                in_=qt_d[d * P:(d + 1) * P,
                                         (qb + 1) * QBLK:(qb + 2) * QBLK])
                        qtbs[qb + 1] = nxq

                    # qhT = Wq.T-tiles @ qT   [h on partitions]
                    qhT = qhp.tile([P, HT, QBLK], F32R, tag="qhT")
                    for ht in range(HT):
                        ps = psm.tile([P, QBLK], F32, tag="mmps")
                        for d in range(DT):
                            nc.tensor.matmul(ps, wq_t[:, d, ht * P:(ht + 1) * P],
                                             qtb[:, d],
                                             start=(d == 0), stop=(d == DT - 1))
                        nc.scalar.activation(qhT[:, ht], ps, AF.Copy)

                    probsT = ptp.tile([P, nkt, QBLK], FP16, tag="probsT")

                    def scores_softmax(qt):
                        sps = pss.tile([P, lkc], F32, tag="scps")
                        for (koff, kw) in kslices:
                            for h in range(HT):
                                nc.tensor.matmul(
                                    sps[:, koff:koff + kw],
                                    qhT[:, h, qt * P:(qt + 1) * P],
                                    khb_slice(h, koff, kw),
                                    start=(h == 0), stop=(h == HT - 1))
                        s_t = smp.tile([P, lkc], F32, tag="s", bufs=2)
                        nc.vector.tensor_add(s_t, sps, biasrep_t)
                        mx = smp.tile([P, 1], F32, tag="mx")
                        nc.vector.reduce_max(mx, s_t, axis=AX.X)
                        negmx = smp.tile([P, 1], F32, tag="negmx")
                        nc.vector.tensor_scalar_mul(negmx, mx, -1.0)
                        p_t = smp.tile([P, lkc], FP16, tag="pp", bufs=2)
                        sume = smp.tile([P, 1], F32, tag="sume")
                        nc.scalar.activation(p_t, s_t, AF.Exp, bias=negmx, scale=1.0,
                                             accum_out=sume)
                        recip = smp.tile([P, 1], F32, tag="recip")
                        nc.vector.reciprocal(recip, sume)
                        nc.vector.tensor_scalar_mul(p_t, p_t, recip)
                        return p_t

                    def transposes(qt, p_t):
                        for kt in range(nkt):
                            tp = psm.tile([P, P], FP16, tag="mmps")
                            nc.tensor.transpose(tp, p_t[:, kt * P:(kt + 1) * P],
                                                identh_t)
                            nc.scalar.activation(
                                probsT[:, kt, qt * P:(qt + 1) * P], tp, AF.Copy)

                    def av_half(outut, half):
                        q0 = half * (QBLK // 2)
                        qw = QBLK // 2
                        for ht in range(HT):
                            ps = psm.tile([P, QBLK], F32, tag="mmps")
                            for kt in range(nkt):
                                nc.tensor.matmul(
                                    ps[:, 0:qw],
                                    vh_t[:, kt, ht * P:(ht + 1) * P],
                                    probsT[:, kt, q0:q0 + qw],
                                    start=(kt == 0), stop=(kt == nkt - 1))
                            nc.scalar.activation(outut[:, ht, q0:q0 + qw],
                                                 ps[:, 0:qw], AF.Copy)

                    # emission order keeps the PE fed across softmax chains
                    p0 = scores_softmax(0)
                    p1 = scores_softmax(1)
                    transposes(0, p0)
                    p2 = scores_softmax(2)
                    transposes(1, p1)
                    p3 = scores_softmax(3)
                    outut = oup.tile([P, HT, QBLK], FP16, tag="outut")
                    av_half(outut, 0)
                    transposes(2, p2)
                    transposes(3, p3)
                    av_half(outut, 1)

                    # final projection (+bo2), write out
                    for vt in range(DT):
                        ps = psm.tile([P, QBLK], F32, tag="mmps")
                        for h in range(HT):
                            nc.tensor.matmul(ps, wo_t[:, h, vt * P:(vt + 1) * P],
                                             outut[:, h],
                                             start=(h == 0), stop=(h == HT - 1))
                        ot = stp.tile([P, QBLK], F32, tag="ot")
                        nc.scalar.activation(ot, ps, AF.Identity, bias=bo2_t[:, vt:vt + 1])
                        nc.sync.dma_start(
                            out=out_d[vt * P:(vt + 1) * P, qb * QBLK:(qb + 1) * QBLK],
                            in_=ot)
    nc.compile()
    return nc


_NC_CACHE = {}


def _get_nc(lkc=LKC_DEFAULT):
    if lkc not in _NC_CACHE:
        _NC_CACHE[lkc] = build_nc(lkc)
    return _NC_CACHE[lkc]


def _pad_up(n, m):
    return ((n + m - 1) // m) * m


def prepare(q, k, mask, Wq, bq, Wk, bk, Wv, bv, Wo, bo):
    """Returns (nc, in_maps) for run_bass_kernel_spmd."""
    q = np.asarray(q, np.float32)
    k = np.asarray(k, np.float32)
    mask = np.asarray(mask)
    Wq = np.asarray(Wq, np.float32)
    Wk = np.asarray(Wk, np.float32)
    Wv = np.asarray(Wv, np.float32)
    Wo = np.asarray(Wo, np.float32)
    bq_ = np.asarray(bq, np.float32)
    bk_ = np.asarray(bk, np.float32)
    bv_ = np.asarray(bv, np.float32)
    bo_ = np.asarray(bo, np.float32)

    nnz_max = int(mask.astype(bool).sum(axis=1).max())
    lkc = max(LKC_DEFAULT, _pad_up(max(nnz_max, 256), 256))
    nc = _get_nc(lkc)

    wq16 = Wq  # stays f32
    wv16 = Wv.astype(np.float16)
    wo16 = Wo.astype(np.float16)
    bo2 = (bv_.astype(np.float64) @ Wo.astype(np.float64) + bo_).astype(np.float32)
    ones = np.ones((1, P), np.float32)
    in_maps = []
    for b in range(B):
        idx = np.nonzero(mask[b])[0]
        ktc = np.zeros((D, lkc), np.float32)
        ktc[:, :len(idx)] = k[b][idx].T
        maskb = np.full((1, lkc), -10000.0, np.float32)
        maskb[0, :len(idx)] = 0.0
        in_maps.append({
            "qt": np.ascontiguousarray(q[b].T),
            "ktc": ktc,
            "wk": Wk, "wq": wq16, "wv": wv16, "wo": wo16,
            "bk": bk_.reshape(H, 1), "bq": bq_.reshape(H, 1),
            "bo2": bo2.reshape(D, 1),
            "maskb": maskb,
            "ones": ones,
        })
    return nc, in_maps


def kernel(q, k, mask, Wq, bq, Wk, bk, Wv, bv, Wo, bo):
    nc, in_maps = prepare(q, k, mask, Wq, bq, Wk, bk, Wv, bv, Wo, bo)
    res = run_bass_kernel_spmd(nc, in_maps, core_ids=list(range(B)))
    out = np.stack([np.ascontiguousarray(res.results[b]["out"].T) for b in range(B)])
    return out.astype(np.float32)


# revision 30
# speedup vs baseline: 1.8887x; 1.1784x over previous
"""nn_Attention Trainium2 Bass kernel — data-parallel over batch on 8 NeuronCores.

Per core (one batch element): full attention
  qh = q@Wq + bq; kh = k@Wk + bk; vh = kh@Wv + bv
  scores = qh@kh.T  (+ mask -> -10000); probs = softmax(scores)
  out = (probs @ vh) @ Wo + bo

Key optimizations vs the naive plan:
  * k-compaction: masked-out k columns (mask==0) contribute nothing, so the
    host gathers only the unmasked columns (~1058 of 2048) and pads to
    LKC=1280.  khT/vh/scores/AV PE work drops by LKC/L = 37.5%.
  * No DRAM spills: khT (f32r) and vh (fp16) stay SBUF-resident; vh and the
    bias row are computed per 512-column block right after that block of khT.
  * qh is projected per q-block inside the main loop (qhT = Wq.T @ qT), so
    scores = qhT.T-tiles @ khT with khT as the wide moving operand.
  * fp16 for the probs/vh/Wo path (same PE speed as bf16, 8x better eps).
    Scores path stays f32r end-to-end.
  * Wq/Wo streamed in 128-col slices just in time (small SBUF footprint).
  * Per q-block emission order interleaves next-qt scores with transposes and
    splits AV into two half-width passes so the PE never waits on softmax.

Device-side algebra (per core):
  khT[h,l]   = Wk-tiles.T @ ktc          (+bk per-partition)      [f32r]
  vh[l,v]    = khT.T-tiles @ Wv          (no bias; bv folded into bo2) [fp16]
  bqRow[k]   = bq.T @ khT ; biasRep = broadcast(maskBias + bqRow)
  per q-block: qhT[h,q] = Wq-tiles.T @ qT                          [f32r]
  per q-tile:  scores = qhT.T-tiles @ khT  [+ biasRep]
               softmax rowwise (max, exp with accum-sum, reciprocal)
               probsT via PE transposes (fp16)
  outUT[h,q] = vh-tiles.T @ probsT                                 [fp16]
  finalT[v,q]= Wo-tiles.T @ outUT       (+bo2 per-partition)       [f32]
Host: out[b] = finalT.T ;  bo2 = bv@Wo + bo  (exact: probs rows sum to 1)
"""
import numpy as np

import concourse.bass as bass
import concourse.mybir as mybir
from concourse import bacc, tile
from concourse.bass_utils import run_bass_kernel_spmd
from concourse.masks import make_identity

B, L, D, H = 8, 2048, 1024, 1024
P = 128
F32 = mybir.dt.float32
F32R = mybir.dt.float32r
FP16 = mybir.dt.float16
AF = mybir.ActivationFunctionType
AX = mybir.AxisListType

QBLK = 1024         # q columns per main-loop block
NQT = QBLK // P     # 8 q tiles per block
NQB = L // QBLK     # 2
DT = D // P         # 8 d tiles
HT = H // P         # 8 h tiles

LKC_DEFAULT = 1280  # padded compact-k length (multiple of 256)


def build_nc(lkc=LKC_DEFAULT):
    assert lkc % 256 == 0
    # k-block slices: all >=256 wide so f32r matmuls stay 1 cycle/row
    kslices = []
    off = 0
    while off < lkc:
        w = 512 if lkc - off >= 512 else lkc - off
        kslices.append((off, w))
        off += w
    nkt = lkc // P  # k tiles of 128
    # AV/probsT cover only the k tiles that can hold unmasked columns; the
    # host guarantees nnz <= lkc - P (cols beyond are pure padding, prob==0)
    nkt_av = nkt - 1

    nc = bacc.Bacc("TRN2", target_bir_lowering=False, debug=False, num_devices=8)
    qt_d = nc.dram_tensor("qt", [D, L], F32R, kind="ExternalInput").ap()
    ktc_d = nc.dram_tensor("ktc", [D, lkc], F32R, kind="ExternalInput").ap()
    wk_d = nc.dram_tensor("wk", [D, H], F32R, kind="ExternalInput").ap()
    wq_d = nc.dram_tensor("wq", [D, H], F32R, kind="ExternalInput").ap()
    wv_d = nc.dram_tensor("wv", [H, H], FP16, kind="ExternalInput").ap()
    wo_d = nc.dram_tensor("wo", [H, D], FP16, kind="ExternalInput").ap()
    bk_d = nc.dram_tensor("bk", [H, 1], F32, kind="ExternalInput").ap()
    bq_d = nc.dram_tensor("bq", [H, 1], F32R, kind="ExternalInput").ap()
    bo2_d = nc.dram_tensor("bo2", [D, 1], F32, kind="ExternalInput").ap()
    maskb_d = nc.dram_tensor("maskb", [1, lkc], F32, kind="ExternalInput").ap()
    ones_d = nc.dram_tensor("ones", [1, P], F32R, kind="ExternalInput").ap()
    out_d = nc.dram_tensor("out", [D, L], F32, kind="ExternalOutput").ap()

    with tile.TileContext(nc) as tc:
        with tc.tile_pool(name="const", bufs=1) as cp, \
             tc.tile_pool(name="persist", bufs=1) as pp_:
            bk_t = cp.tile([P, HT], F32)
            bq_t = cp.tile([P, HT], F32R)
            bo2_t = cp.tile([P, DT], F32)
            for i in range(HT):
                nc.gpsimd.dma_start(out=bk_t[:, i:i + 1], in_=bk_d[i * P:(i + 1) * P, :])
                nc.gpsimd.dma_start(out=bq_t[:, i:i + 1], in_=bq_d[i * P:(i + 1) * P, :])
                nc.gpsimd.dma_start(out=bo2_t[:, i:i + 1], in_=bo2_d[i * P:(i + 1) * P, :])
            onesr_t = cp.tile([1, P], F32R)
            nc.gpsimd.dma_start(out=onesr_t, in_=ones_d)
            ident_f = cp.tile([P, P], F32)
            make_identity(nc, ident_f)
            identh_t = cp.tile([P, P], FP16)
            nc.vector.tensor_copy(identh_t, ident_f)
            biasrep_t = cp.tile([P, lkc], F32)

            # persistent across the whole kernel.  khT is split into two
            # tiles so every 512-col (and the 256-col) moving slice is
            # 2KB-aligned in SBUF (a 5120B row stride would misalign odd-h
            # slices and halve the PE stream rate).
            khbA = pp_.tile([P, HT, 1024], F32R)    # khT k-cols [0,1024)
            khbB = pp_.tile([P, HT, lkc - 1024], F32R)  # khT k-cols [1024,lkc)
            vh_t = pp_.tile([P, nkt_av, H], FP16)      # vh,  k on partitions

            def khb_slice(h, koff, kw):
                if koff < 1024:
                    assert koff + kw <= 1024
                    return khbA[:, h, koff:koff + kw]
                return khbB[:, h, koff - 1024:koff - 1024 + kw]

            # Pre-reserve right-side SBUF for Wq and the first q-block and
            # load them at t~0 on otherwise-idle queues, so phase B starts
            # without a DMA wall (their space never collides with A pools).
            bwq_cm = tc.tile_pool(name="bwq", bufs=1, side="right")
            wqp = bwq_cm.__enter__()
            wq_t = wqp.tile([P, DT, H], F32R, tag="wq")
            bq1_cm = tc.tile_pool(name="bq1", bufs=1, side="right")
            qp1 = bq1_cm.__enter__()
            first_q = qp1.tile([P, DT, QBLK], F32R, tag="qtb0")
            for d in range(DT):
                nc.gpsimd.dma_start(out=first_q[:, d],
                                    in_=qt_d[d * P:(d + 1) * P, 0:QBLK])

            # ---------- A: khT (+bk), vh, bias row — fused per k-block
            with tc.tile_pool(name="aw", bufs=1) as awp, \
                 tc.tile_pool(name="akt", bufs=2) as aktp, \
                 tc.tile_pool(name="akh", bufs=2) as akhp, \
                 tc.tile_pool(name="aps", bufs=2, space="PSUM") as apsp, \
                 tc.tile_pool(name="abps", bufs=1, space="PSUM") as abpsp:
                wk_t = awp.tile([P, DT, H], F32R, tag="wk")
                wv_t = awp.tile([P, HT, H], FP16, tag="wv")
                for i in range(HT):
                    nc.scalar.dma_start(out=wv_t[:, i], in_=wv_d[i * P:(i + 1) * P, :])
                for i in range(DT):
                    nc.scalar.dma_start(out=wq_t[:, i], in_=wq_d[i * P:(i + 1) * P, :])
                for lb, (koff, kw) in enumerate(kslices):
                    ktb = aktp.tile([P, DT, 512], F32R, tag="ktb")
                    for d in range(DT):
                        if lb == 0:
                            nc.sync.dma_start(out=wk_t[:, d],
                                              in_=wk_d[d * P:(d + 1) * P, :])
                        nc.sync.dma_start(out=ktb[:, d, 0:kw],
                                          in_=ktc_d[d * P:(d + 1) * P, koff:koff + kw])
                    khh = akhp.tile([P, HT, 512], FP16, tag="khh")
                    for ht in range(HT):
                        ps = apsp.tile([P, 512], F32, tag="ps")
                        for d in range(DT):
                            nc.tensor.matmul(ps[:, 0:kw],
                                             wk_t[:, d, ht * P:(ht + 1) * P],
                                             ktb[:, d, 0:kw],
                                             start=(d == 0), stop=(d == DT - 1))
                        nc.scalar.activation(khb_slice(ht, koff, kw), ps[:, 0:kw],
                                             AF.Identity, bias=bk_t[:, ht:ht + 1])
                        nc.vector.tensor_copy(khh[:, ht, 0:kw],
                                              khb_slice(ht, koff, kw))
                    # vh for this k-block
                    for lt in range(kw // P):
                        kt_idx = (koff // P) + lt
                        if kt_idx >= nkt_av:
                            continue
                        for vb in range(2):
                            ps = apsp.tile([P, 512], F32, tag="ps")
                            for h in range(HT):
                                nc.tensor.matmul(
                                    ps, khh[:, h, lt * P:(lt + 1) * P],
                                    wv_t[:, h, vb * 512:(vb + 1) * 512],
                                    start=(h == 0), stop=(h == HT - 1))
                            nc.scalar.activation(
                                vh_t[:, kt_idx, vb * 512:(vb + 1) * 512], ps, AF.Copy)
                    # bias row for this k-block: bqRow + mask, replicated
                    ps1 = abpsp.tile([1, 512], F32, tag="bqps")
                    for h in range(HT):
                        nc.tensor.matmul(ps1[:, 0:kw], bq_t[:, h:h + 1],
                                         khb_slice(h, koff, kw),
                                         start=(h == 0), stop=(h == HT - 1))
                    mrow_t = akhp.tile([1, 512], F32, tag="mrow", bufs=1)
                    nc.gpsimd.dma_start(out=mrow_t[:, 0:kw],
                                        in_=maskb_d[:, koff:koff + kw])
                    browr = akhp.tile([1, 512], F32R, tag="browr", bufs=1)
                    nc.scalar.activation(browr[:, 0:kw], ps1[:, 0:kw], AF.Copy)
                    nc.vector.tensor_add(browr[:, 0:kw], browr[:, 0:kw],
                                         mrow_t[:, 0:kw])
                    ps2 = apsp.tile([P, 512], F32, tag="ps")
                    nc.tensor.matmul(ps2[:, 0:kw], onesr_t, browr[:, 0:kw],
                                     start=True, stop=True)
                    nc.scalar.activation(biasrep_t[:, koff:koff + kw], ps2[:, 0:kw],
                                         AF.Copy)

            # ---------- B: attention per q-block
            with tc.tile_pool(name="bq2", bufs=1) as qp, \
                 tc.tile_pool(name="bwq", bufs=1) as wqp, \
                 tc.tile_pool(name="bwo", bufs=1) as wop, \
                 tc.tile_pool(name="bqh", bufs=1) as qhp, \
                 tc.tile_pool(name="bsm", bufs=2) as smp, \
                 tc.tile_pool(name="bpt", bufs=1) as ptp, \
                 tc.tile_pool(name="bst", bufs=2) as stp, \
                 tc.tile_pool(name="bps_s", bufs=2, space="PSUM") as pss, \
                 tc.tile_pool(name="bps_m", bufs=2, space="PSUM") as psm:
                # Wq/Wo resident for all of B; the DMAs reuse SBUF freed by
                # the A-phase weight pools and overlap A's compute tail.
                wq_t = wqp.tile([P, DT, H], F32R, tag="wq")
                wo_t = wop.tile([P, HT, H], FP16, tag="wo")
                for i in range(DT):
                    nc.scalar.dma_start(out=wq_t[:, i], in_=wq_d[i * P:(i + 1) * P, :])
                for i in range(HT):
                    nc.scalar.dma_start(out=wo_t[:, i], in_=wo_d[i * P:(i + 1) * P, :])
                qtbs = {}
                for qb in range(NQB):
                    if qb == 0:
                        qtb = qp.tile([P, DT, QBLK], F32R, tag="qtb")
                        for d in range(DT):
                            nc.sync.dma_start(out=qtb[:, d],
                                              in_=qt_d[d * P:(d + 1) * P, 0:QBLK])
                    else:
                        qtb = qtbs.pop(qb)
                    # prefetch next q-block
                    if qb + 1 < NQB:
                        nxq = qp.tile([P, DT, QBLK], F32R, tag="qtb")
                        for d in range(DT):
                            nc.gpsimd.dma_start(
                                out=nxq[:, d],
                                in_=qt_d[d * P:(d + 1) * P,
                                         (qb + 1) * QBLK:(qb + 2) * QBLK])
                        qtbs[qb + 1] = nxq

                    # qhT = Wq.T-tiles @ qT   [h on partitions], 512-wide
                    qhT = qhp.tile([P, HT, QBLK], F32R, tag="qhT")
                    for ht in range(HT):
                        for qh2 in range(QBLK // 512):
                            q0 = qh2 * 512
                            ps = psm.tile([P, 512], F32, tag="mmps")
                            for d in range(DT):
                                nc.tensor.matmul(ps, wq_t[:, d, ht * P:(ht + 1) * P],
                                                 qtb[:, d, q0:q0 + 512],
                                                 start=(d == 0), stop=(d == DT - 1))
                            nc.scalar.activation(qhT[:, ht, q0:q0 + 512], ps, AF.Copy)

                    probsT = ptp.tile([P, nkt_av, QBLK], FP16, tag="probsT")

                    def scores_softmax(qt):
                        sps = pss.tile([P, lkc], F32, tag="scps")
                        for (koff, kw) in kslices:
                            for h in range(HT):
                                nc.tensor.matmul(
                                    sps[:, koff:koff + kw],
                                    qhT[:, h, qt * P:(qt + 1) * P],
                                    khb_slice(h, koff, kw),
                                    start=(h == 0), stop=(h == HT - 1))
                        s_t = smp.tile([P, lkc], F32, tag="s", bufs=2)
                        nc.vector.tensor_add(s_t, sps, biasrep_t)
                        mx = smp.tile([P, 1], F32, tag="mx")
                        nc.vector.reduce_max(mx, s_t, axis=AX.X)
                        negmx = smp.tile([P, 1], F32, tag="negmx")
                        nc.vector.tensor_scalar_mul(negmx, mx, -1.0)
                        p_t = smp.tile([P, lkc], FP16, tag="pp", bufs=2)
                        sume = smp.tile([P, 1], F32, tag="sume")
                        nc.scalar.activation(p_t, s_t, AF.Exp, bias=negmx, scale=1.0,
                                             accum_out=sume)
                        recip = smp.tile([P, 1], F32, tag="recip")
                        nc.vector.reciprocal(recip, sume)
                        nc.vector.tensor_scalar_mul(p_t, p_t, recip)
                        return p_t

                    def transposes(qt, p_t):
                        for kt in range(nkt_av):
                            tp = psm.tile([P, P], FP16, tag="mmps")
                            nc.tensor.transpose(tp, p_t[:, kt * P:(kt + 1) * P],
                                                identh_t)
                            nc.scalar.activation(
                                probsT[:, kt, qt * P:(qt + 1) * P], tp, AF.Copy)

                    def av_half(outut, half):
                        q0 = half * (QBLK // 2)
                        qw = QBLK // 2
                        for ht in range(HT):
                            ps = psm.tile([P, 512], F32, tag="mmps")
                            for kt in range(nkt_av):
                                nc.tensor.matmul(
                                    ps[:, 0:qw],
                                    vh_t[:, kt, ht * P:(ht + 1) * P],
                                    probsT[:, kt, q0:q0 + qw],
                                    start=(kt == 0), stop=(kt == nkt_av - 1))
                            nc.vector.tensor_copy(outut[:, ht, q0:q0 + qw],
                                                  ps[:, 0:qw])

                    # emission order keeps the PE fed across softmax chains
                    ps_ = {}
                    ps_[0] = scores_softmax(0)
                    ps_[1] = scores_softmax(1)
                    transposes(0, ps_[0])
                    ps_[2] = scores_softmax(2)
                    transposes(1, ps_[1])
                    ps_[3] = scores_softmax(3)
                    transposes(2, ps_[2])
                    ps_[4] = scores_softmax(4)
                    transposes(3, ps_[3])
                    ps_[5] = scores_softmax(5)
                    transposes(4, ps_[4])
                    ps_[6] = scores_softmax(6)
                    transposes(5, ps_[5])
                    ps_[7] = scores_softmax(7)
                    outut = oup.tile([P, HT, QBLK], FP16, tag="outut")
                    av_half(outut, 0)
                    transposes(6, ps_[6])
                    transposes(7, ps_[7])
                    av_half(outut, 1)

                    # final projection (+bo2) in 512-wide halves, write out
                    for vt in range(DT):
                        for fh in range(QBLK // 512):
                            q0 = fh * 512
                            ps = psm.tile([P, 512], F32, tag="mmps")
                            for h in range(HT):
                                nc.tensor.matmul(ps, wo_t[:, h, vt * P:(vt + 1) * P],
                                                 outut[:, h, q0:q0 + 512],
                                                 start=(h == 0), stop=(h == HT - 1))
                            ot = stp.tile([P, 512], F32, tag="ot")
                            nc.scalar.activation(ot, ps, AF.Identity,
                                                 bias=bo2_t[:, vt:vt + 1])
                            nc.sync.dma_start(
                                out=out_d[vt * P:(vt + 1) * P,
                                          qb * QBLK + q0:qb * QBLK + q0 + 512],
                                in_=ot)
            bou_cm.__exit__(None, None, None)
            bwq_cm.__exit__(None, None, None)
    nc.compile()
    return nc


_NC_CACHE = {}


def _get_nc(lkc=LKC_DEFAULT):
    if lkc not in _NC_CACHE:
        _NC_CACHE[lkc] = build_nc(lkc)
    return _NC_CACHE[lkc]


def _pad_up(n, m):
    return ((n + m - 1) // m) * m


def prepare(q, k, mask, Wq, bq, Wk, bk, Wv, bv, Wo, bo):
    """Returns (nc, in_maps) for run_bass_kernel_spmd."""
    q = np.asarray(q, np.float32)
    k = np.asarray(k, np.float32)
    mask = np.asarray(mask)
    Wq = np.asarray(Wq, np.float32)
    Wk = np.asarray(Wk, np.float32)
    Wv = np.asarray(Wv, np.float32)
    Wo = np.asarray(Wo, np.float32)
    bq_ = np.asarray(bq, np.float32)
    bk_ = np.asarray(bk, np.float32)
    bv_ = np.asarray(bv, np.float32)
    bo_ = np.asarray(bo, np.float32)

    nnz_max = int(mask.astype(bool).sum(axis=1).max())
    # the kernel's AV stage skips the last 128-col tile, so padding must
    # guarantee at least 128 pure-pad columns (prob exactly 0 there)
    lkc = max(LKC_DEFAULT, _pad_up(nnz_max + P, 256))
    nc = _get_nc(lkc)

    wq16 = Wq.astype(np.float16)
    wk16 = Wk.astype(np.float16)
    wv16 = Wv.astype(np.float16)
    wo16 = Wo.astype(np.float16)
    bo2 = (bv_.astype(np.float64) @ Wo.astype(np.float64) + bo_).astype(np.float32)
    ones = np.ones((1, P), np.float32)
    in_maps = []
    for b in range(B):
        idx = np.nonzero(mask[b])[0]
        ktc = np.zeros((D, lkc), np.float16)
        ktc[:, :len(idx)] = k[b][idx].T.astype(np.float16)
        maskb = np.full((1, lkc), -10000.0, np.float32)
        maskb[0, :len(idx)] = 0.0
        in_maps.append({
            "qt": np.ascontiguousarray(q[b].T.astype(np.float16)),
            "ktc": ktc,
            "wk": wk16, "wq": wq16, "wv": wv16, "wo": wo16,
            "bk": bk_.reshape(H, 1), "bq": bq_.reshape(H, 1),
            "bo2": bo2.reshape(D, 1),
            "maskb": maskb,
            "ones": ones,
        })
    return nc, in_maps


def kernel(q, k, mask, Wq, bq, Wk, bk, Wv, bv, Wo, bo):
    nc, in_maps = prepare(q, k, mask, Wq, bq, Wk, bk, Wv, bv, Wo, bo)
    res = run_bass_kernel_spmd(nc, in_maps, core_ids=list(range(B)))
    out = np.stack([np.ascontiguousarray(res.results[b]["out"].T) for b in range(B)])
    return out.astype(np.float32)
